# revision 1
# baseline (speedup 1.0000x reference)
"""AOL-Linear (normalization) Trainium2 kernel, 8-core data-parallel.

Math (reference):
    PTP = |P^T @ P|                 # [D, D]
    d   = 1/sqrt(PTP.sum(0) + eps)  # [D]   (PTP symmetric -> row-sum == col-sum)
    W   = P * d[None, :]            # [D, D]
    out = x @ W^T + bias            # [B, D]

Sharding: x row-sharded across 8 cores (4096 rows each). The Gram work is
also split: core c computes rows [c*256, (c+1)*256) of |P^T P|, row-sums
them (symmetry) to get its 256 entries of d, then an AllGather shares d.

Engine-stream layout (every sequencer is in-order, so each stream is kept
free of waits on unrelated downstream work):
  scalar (ACT HWDGE): pc/pk/x loads, x_bf stores, out stores -- out stores
      for block bb are emitted after the x loads of block bb+3 so the x
      pipeline never queues behind an eviction-dependent store.
  sync (SP HWDGE): p_bf stores, all xbar transpose-loads.
  gpsimd: only the tiny d stores/loads and the AllGather, so the
      collective triggers as soon as d is computed.
  DVE: all f32->bf16 casts, gram reduce/recip, W^T scale, PSUM evictions.
  ACT compute: sqrt, bias/d copybacks.
  PE: gram + main matmuls (bf16, N=512), one [16,128] d transpose.

"""

import numpy as np

import concourse.bass as bass
import concourse.mybir as mybir
import concourse.tile as tile
from concourse import bacc
from concourse import bass_utils as _bu
from concourse.bass_utils import run_bass_kernel_spmd
from concourse.masks import make_identity

F32 = mybir.dt.float32
BF16 = mybir.dt.bfloat16

NCORES = 8
D = 2048
B_TOTAL = 32768
B_LOCAL = B_TOTAL // NCORES  # 4096
RC = D // NCORES             # 256 gram rows per core
EPS = 1e-10
P128 = 128

# (walrus --enable-ldw-opt rejects the InstLdweights that Bacc emits, so
# the redundant weight reloads between same-stationary matmuls stay.)


def build_nc(b_local=B_LOCAL):
    KT = D // 128            # 16 contraction tiles
    NT = D // 512            # 4 psum bank slices
    MBLK = min(512, b_local)  # batch rows per transpose-load block
    NBLK = b_local // MBLK
    MSUB = MBLK // 128       # 128-row sub-blocks per block

    nc = bacc.Bacc(None, target_bir_lowering=False, debug=False,
                   num_devices=NCORES, num_swdge_queues=4)

    x_ext = nc.declare_dram_parameter("x", [b_local, D], F32, isOutput=False)
    p_ext = nc.declare_dram_parameter("P", [D, D], F32, isOutput=False)
    pc_ext = nc.declare_dram_parameter("p_cols", [D, RC], F32, isOutput=False)
    bias_ext = nc.declare_dram_parameter("bias", [1, D], F32, isOutput=False)
    out_ext = nc.declare_dram_parameter("out", [b_local, D], F32, isOutput=True)

    with tile.TileContext(nc) as tc:
        with (
            tc.tile_pool(name="dram", bufs=1, space="DRAM") as dram,
            tc.tile_pool(name="consts", bufs=1) as consts,
            tc.tile_pool(name="wtp", bufs=1) as wtp,
            tc.tile_pool(name="pcp", bufs=1) as pcp,
            tc.tile_pool(name="pkp", bufs=3) as pkp,
            tc.tile_pool(name="xfp", bufs=2) as xfp,
            tc.tile_pool(name="xtp", bufs=2) as xtp,
            tc.tile_pool(name="outp", bufs=2) as outp,
            tc.tile_pool(name="psum", bufs=2, space="PSUM") as psump,
        ):
            # ---------------- DRAM scratch ----------------
            x_bf = [dram.tile([MBLK, D], BF16, name=f"x_bf{bb}", tag=f"x_bf{bb}")
                    for bb in range(NBLK)]
            p_bf = dram.tile([D, D], BF16, name="p_bf", tag="p_bf")
            # row m of d_in is this core's d for its k-tile m (partition-major)
            d_in = dram.tile([RC // 128, 128], F32, name="d_in", tag="d_in")
            d_out = dram.tile([KT, 128], F32, name="d_out", tag="d_out")

            def xprep_slab(bb, s):
                """x[128 rows] f32 -> bf16 DRAM scratch (alternating HWDGE
                rings, ACT cast so the DVE d-chain stays clean)."""
                r0 = s * 128
                g0 = bb * MBLK + r0
                eng = nc.sync if (bb * MSUB + s) % 2 == 0 else nc.scalar
                xf = xfp.tile([P128, D], F32, name=f"xf{bb}_{s}", tag="x_f")
                eng.dma_start(out=xf[:, :], in_=x_ext[g0:g0 + 128, :])
                xb = xfp.tile([P128, D], BF16, name=f"xb{bb}_{s}", tag="x_b", bufs=2)
                nc.scalar.copy(out=xb[:, :], in_=xf[:, :])
                eng.dma_start(out=x_bf[bb][r0:r0 + 128, :], in_=xb[:, :])

            # ---------------- phase A: gram (+ bf16 P scratch) ----------------
            pc_t = []
            for k in range(KT):
                pf = pcp.tile([P128, RC], F32, name=f"pcf{k}", tag="pc_f", bufs=2)
                nc.scalar.dma_start(out=pf[:, :], in_=pc_ext[k * 128:(k + 1) * 128, :])
                t = pcp.tile([P128, RC], BF16, name=f"pc{k}", tag=f"pc{k}")
                nc.vector.tensor_copy(t[:, :], pf[:, :])
                pc_t.append(t)

            gps = []
            for m in range(RC // 128):
                g = psump.tile([P128, D], F32, name=f"gps{m}", tag="ps")
                gps.append(g)

            for k in range(KT):
                pk_f = pkp.tile([P128, D], F32, name=f"pkf{k}", tag="pk_f")
                ld_eng = nc.sync if k % 2 == 0 else nc.scalar
                st_eng = nc.scalar if k % 2 == 0 else nc.sync
                ld_eng.dma_start(out=pk_f[:, :], in_=p_ext[k * 128:(k + 1) * 128, :])
                pk_b = pkp.tile([P128, D], BF16, name=f"pkb{k}", tag="pk_b")
                nc.vector.tensor_copy(pk_b[:, :], pk_f[:, :])
                st_eng.dma_start(out=p_bf[k * 128:(k + 1) * 128, :], in_=pk_b[:, :])
                for m in range(RC // 128):
                    for n in range(NT):
                        nc.tensor.matmul(
                            gps[m][:, n * 512:(n + 1) * 512],
                            pc_t[k][:, m * 128:(m + 1) * 128],
                            pk_b[:, n * 512:(n + 1) * 512],
                            start=(k == 0),
                            stop=(k == KT - 1),
                        )
                # prefetch block 0's x slabs through the tail of the gram
                if k >= 8 and k % 2 == 0 and (k - 8) // 2 < MSUB:
                    xprep_slab(0, (k - 8) // 2)

            # W^T transposes depend only on p_bf; emit before the d chain so
            # they overlap the reduce/AllGather window on the sync stream.
            wt = []
            for k in range(KT):
                w = wtp.tile([P128, D], BF16, name=f"wt{k}", tag=f"wt{k}")
                nc.sync.dma_start(out=w[:, :], in_=p_bf[:, k * 128:(k + 1) * 128],
                                  transpose=True)
                wt.append(w)

            # ---------------- d = 1/sqrt(rowsum(|gram|)+eps) ----------------
            eps_t = consts.tile([P128, 1], F32)
            nc.any.memset(eps_t[:, :], EPS)
            s_t = consts.tile([P128, RC // 128], F32)
            q_t = consts.tile([P128, RC // 128], F32)
            dl_t = consts.tile([P128, RC // 128], F32)
            for m in range(RC // 128):
                nc.vector.tensor_reduce(
                    out=s_t[:, m:m + 1], in_=gps[m][:, :],
                    axis=mybir.AxisListType.X, op=mybir.AluOpType.add,
                    apply_absolute_value=True,
                )
                nc.scalar.activation(q_t[:, m:m + 1], s_t[:, m:m + 1],
                                     mybir.ActivationFunctionType.Sqrt,
                                     bias=eps_t[:, 0:1])
                nc.vector.reciprocal(dl_t[:, m:m + 1], q_t[:, m:m + 1])
                nc.gpsimd.dma_start(out=d_in[m:m + 1, :], in_=dl_t[:, m:m + 1])

            nc.gpsimd.collective_compute(
                "AllGather",
                mybir.AluOpType.bypass,
                replica_groups=[list(range(NCORES))],
                ins=[d_in[:, :].opt()],
                outs=[d_out[:, :].opt()],
            )

            # ---------------- bias broadcast tile via K=1 outer product ----------------
            # (PE work with no dependency on d; fills the collective window)
            bias_t = consts.tile([P128, D], F32)
            nc.scalar.dma_start(out=bias_t[0:1, :], in_=bias_ext[:, :])
            ones_t = consts.tile([1, P128], F32)
            nc.any.memset(ones_t[:, :], 1.0)
            bias_ps = psump.tile([P128, D], F32, name="bias_ps", tag="ps")
            for n in range(NT):
                nc.tensor.matmul(bias_ps[:, n * 512:(n + 1) * 512],
                                 ones_t[:, :], bias_t[0:1, n * 512:(n + 1) * 512],
                                 start=True, stop=True)
            nc.scalar.copy(out=bias_t[:, :], in_=bias_ps[:, :])

            # d_out rows are k-tiles; transpose [KT,128] -> d_sb [128, KT]
            # on the PE so no 4-byte-scatter DMA is needed.
            d_ld = consts.tile([KT, 128], F32)
            nc.gpsimd.dma_start(out=d_ld[:, :], in_=d_out[:, :])
            ident = consts.tile([KT, KT], F32)
            make_identity(nc, ident)
            d_ps = psump.tile([P128, D], F32, name="d_ps", tag="ps")
            nc.tensor.transpose(d_ps[:, 0:KT], d_ld[:, :], ident[:, :])
            d_sb = consts.tile([P128, KT], F32)
            nc.scalar.copy(out=d_sb[:, :], in_=d_ps[:, 0:KT])

            for k in range(KT):
                nc.vector.tensor_scalar_mul(wt[k][:, :], wt[k][:, :],
                                            d_sb[:, k:k + 1])

            # ---------------- phase B: main matmul (software-pipelined emission) ----------------
            def transposes(bb):
                xt = []
                for k in range(KT):
                    t = xtp.tile([P128, MBLK], BF16, name=f"xt{bb}_{k}", tag=f"xt{k}")
                    nc.sync.dma_start(out=t[:, :],
                                      in_=x_bf[bb][:, k * 128:(k + 1) * 128],
                                      transpose=True)
                    xt.append(t)
                return xt

            for bb in (1, 2):
                if bb < NBLK:
                    for s in range(MSUB):
                        xprep_slab(bb, s)
            xt_cur = transposes(0)

            for bb in range(NBLK):
                xt_next = transposes(bb + 1) if bb + 1 < NBLK else None
                if bb + 3 < NBLK:
                    for s in range(MSUB):
                        xprep_slab(bb + 3, s)
                for m in range(MSUB):
                    ps = psump.tile([P128, D], F32, name=f"ps{bb}_{m}", tag="ps")
                    for k in range(KT):
                        for n in range(NT):
                            nc.tensor.matmul(
                                ps[:, n * 512:(n + 1) * 512],
                                xt_cur[k][:, m * 128:(m + 1) * 128],
                                wt[k][:, n * 512:(n + 1) * 512],
                                start=(k == 0),
                                stop=(k == KT - 1),
                            )
                    r0 = bb * MBLK + m * 128
                    for h in range(2):
                        hd = D // 2
                        ot = outp.tile([P128, hd], F32, name=f"ot{bb}_{m}_{h}",
                                       tag="ot", bufs=3)
                        nc.vector.tensor_add(out=ot[:, :],
                                             in0=ps[:, h * hd:(h + 1) * hd],
                                             in1=bias_t[:, h * hd:(h + 1) * hd])
                        oeng = nc.sync if (m * 2 + h) % 2 == 0 else nc.scalar
                        oeng.dma_start(out=out_ext[r0:r0 + 128, h * hd:(h + 1) * hd],
                                       in_=ot[:, :])
                xt_cur = xt_next

    nc.compile()
    return nc


_NC_CACHE = {}


def _get_nc(b_local=B_LOCAL):
    if b_local not in _NC_CACHE:
        _NC_CACHE[b_local] = build_nc(b_local)
    return _NC_CACHE[b_local]


def make_in_maps(x, P, bias, b_local=B_LOCAL):
    x = np.ascontiguousarray(np.asarray(x, dtype=np.float32))
    P = np.ascontiguousarray(np.asarray(P, dtype=np.float32))
    bias = np.ascontiguousarray(np.asarray(bias, dtype=np.float32)).reshape(1, D)
    in_maps = []
    for c in range(NCORES):
        in_maps.append({
            "x": np.ascontiguousarray(x[c * b_local:(c + 1) * b_local]),
            "P": P,
            "p_cols": np.ascontiguousarray(P[:, c * RC:(c + 1) * RC]),
            "bias": bias,
        })
    return in_maps


def run(x, P, bias, trace=False, b_local=B_LOCAL):
    nc = _get_nc(b_local)
    in_maps = make_in_maps(x, P, bias, b_local)
    res = run_bass_kernel_spmd(nc, in_maps, list(range(NCORES)), trace=trace)
    out = np.concatenate([res.results[c]["out"] for c in range(NCORES)], axis=0)
    return out, res


def kernel(x, P, bias):
    out, _ = run(x, P, bias)
    return np.asarray(out, dtype=np.float32)



# revision 8
# speedup vs baseline: 1.0210x; 1.0210x over previous
"""AOL-Linear (normalization) Trainium2 kernel, 8-core data-parallel.

Math (reference):
    PTP = |P^T @ P|                 # [D, D]
    d   = 1/sqrt(PTP.sum(0) + eps)  # [D]   (PTP symmetric -> row-sum == col-sum)
    W   = P * d[None, :]            # [D, D]
    out = x @ W^T + bias            # [B, D]

Sharding: x row-sharded across 8 cores (4096 rows each). The Gram work is
also split: core c computes rows [c*256, (c+1)*256) of |P^T P|, row-sums
them (symmetry) to get its 256 entries of d, then an AllGather shares d.

The host passes P twice: natural layout (gram moving operand) and
transposed `pT` (W^T tiles) so no on-device transpose of P is needed --
wt tiles stream in as plain contiguous loads concurrently with the gram.
x still needs an on-device transpose (f32 load -> bf16 cast -> DRAM
scratch -> xbar transpose-load); those transpose-loads alternate between
the two HWDGE queues so neither queue serializes the main loop.

Engine-stream layout (every sequencer is in-order):
  scalar (ACT HWDGE): odd-k loads/stores, xb casts, out stores (odd).
  sync (SP HWDGE): even-k loads/stores, transpose-loads (even), out (even).
  gpsimd: tiny d stores/loads + AllGather only, so the collective fires
      as soon as d is computed.
  DVE: pk/pT/pc casts, gram reduce/recip, wt scale, PSUM evictions.
  ACT compute: sqrt, bias/d copybacks, xb casts.
  PE: gram + main matmuls (bf16, N=512), one [16,128] d transpose.
"""

import numpy as np

import concourse.bass as bass
import concourse.mybir as mybir
import concourse.tile as tile
from concourse import bacc
from concourse import bass_utils as _bu
from concourse.bass_utils import run_bass_kernel_spmd
from concourse.masks import make_identity

F32 = mybir.dt.float32
BF16 = mybir.dt.bfloat16

NCORES = 8
D = 2048
B_TOTAL = 32768
B_LOCAL = B_TOTAL // NCORES  # 4096
RC = D // NCORES             # 256 gram rows per core
EPS = 1e-10
P128 = 128

# (walrus --enable-ldw-opt rejects the InstLdweights that Bacc emits, so
# the redundant weight reloads between same-stationary matmuls stay.)


def build_nc(b_local=B_LOCAL):
    KT = D // 128            # 16 contraction tiles
    NT = D // 512            # 4 psum bank slices
    MBLK = min(512, b_local)  # batch rows per transpose-load block
    NBLK = b_local // MBLK
    MSUB = MBLK // 128       # 128-row sub-blocks per block

    nc = bacc.Bacc(None, target_bir_lowering=False, debug=False,
                   num_devices=NCORES, num_swdge_queues=4)

    x_ext = nc.declare_dram_parameter("x", [b_local, D], F32, isOutput=False)
    p_ext = nc.declare_dram_parameter("P", [D, D], F32, isOutput=False)
    pt_ext = nc.declare_dram_parameter("pT", [D, D], F32, isOutput=False)
    pc_ext = nc.declare_dram_parameter("p_cols", [D, RC], F32, isOutput=False)
    bias_ext = nc.declare_dram_parameter("bias", [1, D], F32, isOutput=False)
    out_ext = nc.declare_dram_parameter("out", [b_local, D], F32, isOutput=True)

    with tile.TileContext(nc) as tc:
        with (
            tc.tile_pool(name="dram", bufs=1, space="DRAM") as dram,
            tc.tile_pool(name="consts", bufs=1) as consts,
            tc.tile_pool(name="wtp", bufs=1) as wtp,
            tc.tile_pool(name="pcp", bufs=1) as pcp,
            tc.tile_pool(name="pkp", bufs=2) as pkp,
            tc.tile_pool(name="ptp", bufs=2) as ptp,
            tc.tile_pool(name="xfp", bufs=2) as xfp,
            tc.tile_pool(name="xtp", bufs=2) as xtp,
            tc.tile_pool(name="outp", bufs=2) as outp,
            tc.tile_pool(name="psum", bufs=2, space="PSUM") as psump,
        ):
            # ---------------- DRAM scratch ----------------
            x_bf = [dram.tile([MBLK, D], BF16, name=f"x_bf{bb}", tag=f"x_bf{bb}")
                    for bb in range(NBLK)]
            # row m of d_in is this core's d for its k-tile m (partition-major)
            d_in = dram.tile([RC // 128, 128], F32, name="d_in", tag="d_in")
            d_out = dram.tile([KT, 128], F32, name="d_out", tag="d_out")

            def xprep_slab(bb, s):
                """x[128 rows] f32 -> bf16 DRAM scratch (alternating HWDGE
                rings, ACT cast so the DVE d-chain stays clean)."""
                r0 = s * 128
                g0 = bb * MBLK + r0
                eng = nc.sync if (bb * MSUB + s) % 2 == 0 else nc.scalar
                xf = xfp.tile([P128, D], F32, name=f"xf{bb}_{s}", tag="x_f")
                eng.dma_start(out=xf[:, :], in_=x_ext[g0:g0 + 128, :])
                xb = xfp.tile([P128, D], BF16, name=f"xb{bb}_{s}", tag="x_b", bufs=2)
                nc.scalar.copy(out=xb[:, :], in_=xf[:, :])
                eng.dma_start(out=x_bf[bb][r0:r0 + 128, :], in_=xb[:, :])

            # ---------------- phase A: gram + W^T tile streaming ----------------
            # p_cols (gram stationary) tiles
            pc_t = []
            for k in range(KT):
                pf = pcp.tile([P128, RC], F32, name=f"pcf{k}", tag="pc_f", bufs=2)
                nc.scalar.dma_start(out=pf[:, :], in_=pc_ext[k * 128:(k + 1) * 128, :])
                t = pcp.tile([P128, RC], BF16, name=f"pc{k}", tag=f"pc{k}")
                nc.vector.tensor_copy(t[:, :], pf[:, :])
                pc_t.append(t)

            gps = []
            for m in range(RC // 128):
                g = psump.tile([P128, D], F32, name=f"gps{m}", tag="ps")
                gps.append(g)

            # W^T tiles (persistent) -- loaded from host-transposed pT, cast
            # to bf16, later scaled in place by d.
            wt = []
            for k in range(KT):
                w = wtp.tile([P128, D], BF16, name=f"wt{k}", tag=f"wt{k}")
                wt.append(w)

            # x blocks 0/1 stream through the gram phase: two slabs up
            # front, one more after every other pk/pT load pair.
            pending = [(bb, s) for bb in (0, 1) if bb < NBLK
                       for s in range(MSUB)]
            for _ in range(2):
                if pending:
                    xprep_slab(*pending.pop(0))

            for k in range(KT):
                ld_eng = nc.sync if k % 2 == 0 else nc.scalar
                alt_eng = nc.scalar if k % 2 == 0 else nc.sync
                pk_f = pkp.tile([P128, D], F32, name=f"pkf{k}", tag="pk_f")
                ld_eng.dma_start(out=pk_f[:, :], in_=p_ext[k * 128:(k + 1) * 128, :])
                pk_b = pkp.tile([P128, D], BF16, name=f"pkb{k}", tag="pk_b")
                nc.vector.tensor_copy(pk_b[:, :], pk_f[:, :])
                pt_f = ptp.tile([P128, D], F32, name=f"ptf{k}", tag="pt_f")
                alt_eng.dma_start(out=pt_f[:, :], in_=pt_ext[k * 128:(k + 1) * 128, :])
                nc.vector.tensor_copy(wt[k][:, :], pt_f[:, :])
                for m in range(RC // 128):
                    for n in range(NT):
                        nc.tensor.matmul(
                            gps[m][:, n * 512:(n + 1) * 512],
                            pc_t[k][:, m * 128:(m + 1) * 128],
                            pk_b[:, n * 512:(n + 1) * 512],
                            start=(k == 0),
                            stop=(k == KT - 1),
                        )
                # keep x block 0/1 streaming through the gram phase
                if k % 2 == 1 and pending:
                    xprep_slab(*pending.pop(0))

            # ---------------- d = 1/sqrt(rowsum(|gram|)+eps) ----------------
            eps_t = consts.tile([P128, 1], F32)
            nc.any.memset(eps_t[:, :], EPS)
            s_t = consts.tile([P128, RC // 128], F32)
            q_t = consts.tile([P128, RC // 128], F32)
            dl_t = consts.tile([P128, RC // 128], F32)
            for m in range(RC // 128):
                nc.vector.tensor_reduce(
                    out=s_t[:, m:m + 1], in_=gps[m][:, :],
                    axis=mybir.AxisListType.X, op=mybir.AluOpType.add,
                    apply_absolute_value=True,
                )
                nc.scalar.activation(q_t[:, m:m + 1], s_t[:, m:m + 1],
                                     mybir.ActivationFunctionType.Sqrt,
                                     bias=eps_t[:, 0:1])
                nc.vector.reciprocal(dl_t[:, m:m + 1], q_t[:, m:m + 1])
                nc.gpsimd.dma_start(out=d_in[m:m + 1, :], in_=dl_t[:, m:m + 1])

            nc.gpsimd.collective_compute(
                "AllGather",
                mybir.AluOpType.bypass,
                replica_groups=[list(range(NCORES))],
                ins=[d_in[:, :].opt()],
                outs=[d_out[:, :].opt()],
            )

            # finish prepping x blocks 0/1 if the gram loop didn't
            while pending:
                xprep_slab(*pending.pop(0))

            # ---------------- bias broadcast tile via K=1 outer product ----------------
            # (PE work with no dependency on d; fills the collective window)
            bias_t = consts.tile([P128, D], F32)
            nc.scalar.dma_start(out=bias_t[0:1, :], in_=bias_ext[:, :])
            ones_t = consts.tile([1, P128], F32)
            nc.any.memset(ones_t[:, :], 1.0)
            bias_ps = psump.tile([P128, D], F32, name="bias_ps", tag="ps")
            for n in range(NT):
                nc.tensor.matmul(bias_ps[:, n * 512:(n + 1) * 512],
                                 ones_t[:, :], bias_t[0:1, n * 512:(n + 1) * 512],
                                 start=True, stop=True)
            nc.scalar.copy(out=bias_t[:, :], in_=bias_ps[:, :])

            # x blocks 2/3 prep during the collective window
            for bb in (2, 3):
                if bb < NBLK:
                    for s in range(MSUB):
                        xprep_slab(bb, s)

            # d_out rows are k-tiles; transpose [KT,128] -> d_sb [128, KT]
            # on the PE so no 4-byte-scatter DMA is needed.
            d_ld = consts.tile([KT, 128], F32)
            nc.gpsimd.dma_start(out=d_ld[:, :], in_=d_out[:, :])
            ident = consts.tile([KT, KT], F32)
            make_identity(nc, ident)
            d_ps = psump.tile([P128, D], F32, name="d_ps", tag="ps")
            nc.tensor.transpose(d_ps[:, 0:KT], d_ld[:, :], ident[:, :])
            d_sb = consts.tile([P128, KT], F32)
            nc.scalar.copy(out=d_sb[:, :], in_=d_ps[:, 0:KT])

            for k in range(KT):
                nc.vector.tensor_scalar_mul(wt[k][:, :], wt[k][:, :],
                                            d_sb[:, k:k + 1])

            # ---------------- phase B: main matmul (software-pipelined emission) ----------------
            def transposes(bb):
                xt = []
                for k in range(KT):
                    t = xtp.tile([P128, MBLK], BF16, name=f"xt{bb}_{k}", tag=f"xt{k}")
                    # all transposes on one queue: concurrent XBAR
                    # transposes from both HWDGE rings corrupt data.
                    nc.sync.dma_start(out=t[:, :],
                                      in_=x_bf[bb][:, k * 128:(k + 1) * 128],
                                      transpose=True)
                    xt.append(t)
                return xt

            xt_cur = transposes(0)

            for bb in range(NBLK):
                xt_next = transposes(bb + 1) if bb + 1 < NBLK else None
                if bb + 4 < NBLK:
                    for s in range(MSUB):
                        xprep_slab(bb + 4, s)
                for m in range(MSUB):
                    ps = psump.tile([P128, D], F32, name=f"ps{bb}_{m}", tag="ps")
                    for k in range(KT):
                        for n in range(NT):
                            nc.tensor.matmul(
                                ps[:, n * 512:(n + 1) * 512],
                                xt_cur[k][:, m * 128:(m + 1) * 128],
                                wt[k][:, n * 512:(n + 1) * 512],
                                start=(k == 0),
                                stop=(k == KT - 1),
                            )
                    r0 = bb * MBLK + m * 128
                    for h in range(2):
                        hd = D // 2
                        ot = outp.tile([P128, hd], F32, name=f"ot{bb}_{m}_{h}",
                                       tag="ot", bufs=3)
                        nc.vector.tensor_add(out=ot[:, :],
                                             in0=ps[:, h * hd:(h + 1) * hd],
                                             in1=bias_t[:, h * hd:(h + 1) * hd])
                        oeng = nc.sync if (m * 2 + h) % 2 == 0 else nc.scalar
                        oeng.dma_start(out=out_ext[r0:r0 + 128, h * hd:(h + 1) * hd],
                                       in_=ot[:, :])
                xt_cur = xt_next

    nc.compile()
    return nc


_NC_CACHE = {}


def _get_nc(b_local=B_LOCAL):
    if b_local not in _NC_CACHE:
        _NC_CACHE[b_local] = build_nc(b_local)
    return _NC_CACHE[b_local]


def make_in_maps(x, P, bias, b_local=B_LOCAL):
    x = np.ascontiguousarray(np.asarray(x, dtype=np.float32))
    P = np.ascontiguousarray(np.asarray(P, dtype=np.float32))
    pT = np.ascontiguousarray(P.T)
    bias = np.ascontiguousarray(np.asarray(bias, dtype=np.float32)).reshape(1, D)
    in_maps = []
    for c in range(NCORES):
        in_maps.append({
            "x": np.ascontiguousarray(x[c * b_local:(c + 1) * b_local]),
            "P": P,
            "pT": pT,
            "p_cols": np.ascontiguousarray(P[:, c * RC:(c + 1) * RC]),
            "bias": bias,
        })
    return in_maps


def run(x, P, bias, trace=False, b_local=B_LOCAL):
    nc = _get_nc(b_local)
    in_maps = make_in_maps(x, P, bias, b_local)
    res = run_bass_kernel_spmd(nc, in_maps, list(range(NCORES)), trace=trace)
    out = np.concatenate([res.results[c]["out"] for c in range(NCORES)], axis=0)
    return out, res


def kernel(x, P, bias):
    out, _ = run(x, P, bias)
    return np.asarray(out, dtype=np.float32)


# revision 9
# speedup vs baseline: 1.0689x; 1.0469x over previous
"""AOL-Linear (normalization) Trainium2 kernel, 8-core data-parallel.

Math (reference):
    PTP = |P^T @ P|                 # [D, D]
    d   = 1/sqrt(PTP.sum(0) + eps)  # [D]   (PTP symmetric -> row-sum == col-sum)
    W   = P * d[None, :]            # [D, D]
    out = x @ W^T + bias            # [B, D]

Sharding: x row-sharded across 8 cores (4096 rows each). The Gram work is
also split: core c computes rows [c*256, (c+1)*256) of |P^T P|, row-sums
them (symmetry) to get its 256 entries of d, then an AllGather shares d.

The host passes P twice: natural layout (gram moving operand) and
transposed `pT` (W^T tiles) so no on-device transpose of P is needed.
x needs an on-device transpose (f32 load -> bf16 cast -> DRAM scratch ->
xbar transpose-load). All transpose-loads stay on ONE queue (concurrent
XBAR transposes from both HWDGE rings corrupt data).

Emission order is priority order for the in-order queues:
  1. P + p_cols loads only -> gram finishes ~50us, d + AllGather fire.
  2. pT loads + x block-0 prep fill the AllGather window.
  3. main loop: per block, prep block bb+1, transpose-load bb+1 (sync),
     matmuls, PSUM evict (DVE) -> out stores on scalar ONLY so they are
     never stuck behind the next block's transposes on sync.
"""

import numpy as np

import concourse.bass as bass
import concourse.mybir as mybir
import concourse.tile as tile
from concourse import bacc
from concourse import bass_utils as _bu
from concourse.bass_utils import run_bass_kernel_spmd
from concourse.masks import make_identity

F32 = mybir.dt.float32
BF16 = mybir.dt.bfloat16

NCORES = 8
D = 2048
B_TOTAL = 32768
B_LOCAL = B_TOTAL // NCORES  # 4096
RC = D // NCORES             # 256 gram rows per core
EPS = 1e-10
P128 = 128

# (walrus --enable-ldw-opt rejects the InstLdweights that Bacc emits, so
# the redundant weight reloads between same-stationary matmuls stay.)


def build_nc(b_local=B_LOCAL):
    KT = D // 128            # 16 contraction tiles
    NT = D // 512            # 4 psum bank slices
    MBLK = min(512, b_local)  # batch rows per transpose-load block
    NBLK = b_local // MBLK
    MSUB = MBLK // 128       # 128-row sub-blocks per block

    nc = bacc.Bacc(None, target_bir_lowering=False, debug=False,
                   num_devices=NCORES, num_swdge_queues=4)

    x_ext = nc.declare_dram_parameter("x", [b_local, D], F32, isOutput=False)
    p_ext = nc.declare_dram_parameter("P", [D, D], F32, isOutput=False)
    pt_ext = nc.declare_dram_parameter("pT", [D, D], F32, isOutput=False)
    pc_ext = nc.declare_dram_parameter("p_cols", [D, RC], F32, isOutput=False)
    bias_ext = nc.declare_dram_parameter("bias", [1, D], F32, isOutput=False)
    out_ext = nc.declare_dram_parameter("out", [b_local, D], F32, isOutput=True)

    with tile.TileContext(nc) as tc:
        with (
            tc.tile_pool(name="dram", bufs=1, space="DRAM") as dram,
            tc.tile_pool(name="consts", bufs=1) as consts,
            tc.tile_pool(name="wtp", bufs=1) as wtp,
            tc.tile_pool(name="pcp", bufs=1) as pcp,
            tc.tile_pool(name="pkp", bufs=2) as pkp,
            tc.tile_pool(name="ptp", bufs=2) as ptp,
            tc.tile_pool(name="xfp", bufs=2) as xfp,
            tc.tile_pool(name="xtp", bufs=2) as xtp,
            tc.tile_pool(name="outp", bufs=2) as outp,
            tc.tile_pool(name="psum", bufs=2, space="PSUM") as psump,
        ):
            # ---------------- DRAM scratch ----------------
            x_bf = [dram.tile([MBLK, D], BF16, name=f"x_bf{bb}", tag=f"x_bf{bb}")
                    for bb in range(NBLK)]
            # row m of d_in is this core's d for its k-tile m (partition-major)
            d_in = dram.tile([RC // 128, 128], F32, name="d_in", tag="d_in")
            d_out = dram.tile([KT, 128], F32, name="d_out", tag="d_out")

            def xprep_slab(bb, s):
                """x[128 rows] f32 -> bf16 DRAM scratch (alternating HWDGE
                rings, ACT cast so the DVE d-chain stays clean)."""
                r0 = s * 128
                g0 = bb * MBLK + r0
                eng = nc.sync if (bb * MSUB + s) % 2 == 0 else nc.scalar
                xf = xfp.tile([P128, D], F32, name=f"xf{bb}_{s}", tag="x_f")
                eng.dma_start(out=xf[:, :], in_=x_ext[g0:g0 + 128, :])
                xb = xfp.tile([P128, D], BF16, name=f"xb{bb}_{s}", tag="x_b", bufs=2)
                nc.scalar.copy(out=xb[:, :], in_=xf[:, :])
                eng.dma_start(out=x_bf[bb][r0:r0 + 128, :], in_=xb[:, :])

            def transposes(bb):
                xt = []
                for k in range(KT):
                    t = xtp.tile([P128, MBLK], BF16, name=f"xt{bb}_{k}", tag=f"xt{k}")
                    # all transposes on one queue: concurrent XBAR
                    # transposes from both HWDGE rings corrupt data.
                    nc.sync.dma_start(out=t[:, :],
                                      in_=x_bf[bb][:, k * 128:(k + 1) * 128],
                                      transpose=True)
                    xt.append(t)
                return xt

            # ---------------- phase A1: gram only (P critical path) ----------------
            pc_t = []
            for k in range(KT):
                pf = pcp.tile([P128, RC], F32, name=f"pcf{k}", tag="pc_f", bufs=2)
                nc.scalar.dma_start(out=pf[:, :], in_=pc_ext[k * 128:(k + 1) * 128, :])
                t = pcp.tile([P128, RC], BF16, name=f"pc{k}", tag=f"pc{k}")
                nc.vector.tensor_copy(t[:, :], pf[:, :])
                pc_t.append(t)

            gps = []
            for m in range(RC // 128):
                g = psump.tile([P128, D], F32, name=f"gps{m}", tag="ps")
                gps.append(g)

            wt = []
            for k in range(KT):
                w = wtp.tile([P128, D], BF16, name=f"wt{k}", tag=f"wt{k}")
                wt.append(w)

            for k in range(KT):
                ld_eng = nc.sync if k % 2 == 0 else nc.scalar
                pk_f = pkp.tile([P128, D], F32, name=f"pkf{k}", tag="pk_f")
                ld_eng.dma_start(out=pk_f[:, :], in_=p_ext[k * 128:(k + 1) * 128, :])
                pk_b = pkp.tile([P128, D], BF16, name=f"pkb{k}", tag="pk_b")
                nc.vector.tensor_copy(pk_b[:, :], pk_f[:, :])
                for m in range(RC // 128):
                    for n in range(NT):
                        nc.tensor.matmul(
                            gps[m][:, n * 512:(n + 1) * 512],
                            pc_t[k][:, m * 128:(m + 1) * 128],
                            pk_b[:, n * 512:(n + 1) * 512],
                            start=(k == 0),
                            stop=(k == KT - 1),
                        )

            # ---------------- d = 1/sqrt(rowsum(|gram|)+eps) ----------------
            eps_t = consts.tile([P128, 1], F32)
            nc.any.memset(eps_t[:, :], EPS)
            s_t = consts.tile([P128, RC // 128], F32)
            q_t = consts.tile([P128, RC // 128], F32)
            dl_t = consts.tile([P128, RC // 128], F32)
            for m in range(RC // 128):
                nc.vector.tensor_reduce(
                    out=s_t[:, m:m + 1], in_=gps[m][:, :],
                    axis=mybir.AxisListType.X, op=mybir.AluOpType.add,
                    apply_absolute_value=True,
                )
                nc.scalar.activation(q_t[:, m:m + 1], s_t[:, m:m + 1],
                                     mybir.ActivationFunctionType.Sqrt,
                                     bias=eps_t[:, 0:1])
                nc.vector.reciprocal(dl_t[:, m:m + 1], q_t[:, m:m + 1])
                nc.gpsimd.dma_start(out=d_in[m:m + 1, :], in_=dl_t[:, m:m + 1])

            nc.gpsimd.collective_compute(
                "AllGather",
                mybir.AluOpType.bypass,
                replica_groups=[list(range(NCORES))],
                ins=[d_in[:, :].opt()],
                outs=[d_out[:, :].opt()],
            )

            # ---------------- phase A2: pT stream + x block 0 (AllGather window) ----------------
            pend0 = [(0, s) for s in range(MSUB)]
            for k in range(KT):
                alt_eng = nc.scalar if k % 2 == 0 else nc.sync
                pt_f = ptp.tile([P128, D], F32, name=f"ptf{k}", tag="pt_f")
                alt_eng.dma_start(out=pt_f[:, :], in_=pt_ext[k * 128:(k + 1) * 128, :])
                nc.vector.tensor_copy(wt[k][:, :], pt_f[:, :])
                if k % 4 == 1 and pend0:
                    xprep_slab(*pend0.pop(0))
            while pend0:
                xprep_slab(*pend0.pop(0))

            # bias broadcast tile via K=1 outer product (PE work with no
            # dependency on d; fills the collective window)
            bias_t = consts.tile([P128, D], F32)
            nc.scalar.dma_start(out=bias_t[0:1, :], in_=bias_ext[:, :])
            ones_t = consts.tile([1, P128], F32)
            nc.any.memset(ones_t[:, :], 1.0)
            bias_ps = psump.tile([P128, D], F32, name="bias_ps", tag="ps")
            for n in range(NT):
                nc.tensor.matmul(bias_ps[:, n * 512:(n + 1) * 512],
                                 ones_t[:, :], bias_t[0:1, n * 512:(n + 1) * 512],
                                 start=True, stop=True)
            nc.scalar.copy(out=bias_t[:, :], in_=bias_ps[:, :])

            # d_out rows are k-tiles; transpose [KT,128] -> d_sb [128, KT]
            # on the PE so no 4-byte-scatter DMA is needed.
            d_ld = consts.tile([KT, 128], F32)
            nc.gpsimd.dma_start(out=d_ld[:, :], in_=d_out[:, :])
            ident = consts.tile([KT, KT], F32)
            make_identity(nc, ident)
            d_ps = psump.tile([P128, D], F32, name="d_ps", tag="ps")
            nc.tensor.transpose(d_ps[:, 0:KT], d_ld[:, :], ident[:, :])
            d_sb = consts.tile([P128, KT], F32)
            nc.scalar.copy(out=d_sb[:, :], in_=d_ps[:, 0:KT])

            for k in range(KT):
                nc.vector.tensor_scalar_mul(wt[k][:, :], wt[k][:, :],
                                            d_sb[:, k:k + 1])

            xt_cur = transposes(0)

            # ---------------- phase B: main matmul (software-pipelined emission) ----------------
            for bb in range(NBLK):
                if bb + 1 < NBLK:
                    for s in range(MSUB):
                        xprep_slab(bb + 1, s)
                    xt_next = transposes(bb + 1)
                else:
                    xt_next = None
                for m in range(MSUB):
                    ps = psump.tile([P128, D], F32, name=f"ps{bb}_{m}", tag="ps")
                    for k in range(KT):
                        for n in range(NT):
                            nc.tensor.matmul(
                                ps[:, n * 512:(n + 1) * 512],
                                xt_cur[k][:, m * 128:(m + 1) * 128],
                                wt[k][:, n * 512:(n + 1) * 512],
                                start=(k == 0),
                                stop=(k == KT - 1),
                            )
                    r0 = bb * MBLK + m * 128
                    for h in range(2):
                        hd = D // 2
                        ot = outp.tile([P128, hd], F32, name=f"ot{bb}_{m}_{h}",
                                       tag="ot", bufs=4)
                        nc.vector.tensor_add(out=ot[:, :],
                                             in0=ps[:, h * hd:(h + 1) * hd],
                                             in1=bias_t[:, h * hd:(h + 1) * hd])
                        # out stores all on scalar: never queue an
                        # eviction-gating store behind sync transposes.
                        nc.scalar.dma_start(
                            out=out_ext[r0:r0 + 128, h * hd:(h + 1) * hd],
                            in_=ot[:, :])
                xt_cur = xt_next

    nc.compile()
    return nc


_NC_CACHE = {}


def _get_nc(b_local=B_LOCAL):
    if b_local not in _NC_CACHE:
        _NC_CACHE[b_local] = build_nc(b_local)
    return _NC_CACHE[b_local]


def make_in_maps(x, P, bias, b_local=B_LOCAL):
    x = np.ascontiguousarray(np.asarray(x, dtype=np.float32))
    P = np.ascontiguousarray(np.asarray(P, dtype=np.float32))
    pT = np.ascontiguousarray(P.T)
    bias = np.ascontiguousarray(np.asarray(bias, dtype=np.float32)).reshape(1, D)
    in_maps = []
    for c in range(NCORES):
        in_maps.append({
            "x": np.ascontiguousarray(x[c * b_local:(c + 1) * b_local]),
            "P": P,
            "pT": pT,
            "p_cols": np.ascontiguousarray(P[:, c * RC:(c + 1) * RC]),
            "bias": bias,
        })
    return in_maps


def run(x, P, bias, trace=False, b_local=B_LOCAL):
    nc = _get_nc(b_local)
    in_maps = make_in_maps(x, P, bias, b_local)
    res = run_bass_kernel_spmd(nc, in_maps, list(range(NCORES)), trace=trace)
    out = np.concatenate([res.results[c]["out"] for c in range(NCORES)], axis=0)
    return out, res


def kernel(x, P, bias):
    out, _ = run(x, P, bias)
    return np.asarray(out, dtype=np.float32)
